# revision 41
# baseline (speedup 1.0000x reference)
"""ConvLSTM (pixel-wise, 1x1 convs) Trainium2 Bass kernel.

Math (after exact algebraic folding):
  per pixel, per t:  g1 = W1x @ x_t + W1h @ h1 + b1   (W1x = Wih1 @ (W_red * denorm_scale))
                     i,f,g,o = split(g1); c1 = sig(f)*c1 + sig(i)*tanh(g); h1 = sig(o)*tanh(c1)
                     g2 = W21 @ h1 + W22 @ h2 + b2    (W21 = Wih2 @ Wc1)
                     c2,h2 analogous
  out = (W_head @ Wc2) @ h2_final + const

Sharding: batch b -> core b (8 cores, no collectives).

Per-core layout:
  S1 [92, CHUNK]   rows 0:64 = h1, rows 64:92 = x(t)    (matmul rhs, K=92)
  S2 [128, CHUNK]  rows 0:64 = h1 (dup), 64:128 = h2    (matmul rhs, K=128)
  c1/c2 [128, HALF] : A-half pixels on partitions 0:64, B-half on 64:128
  gate planes [128, FD] in PSUM: per-gate, A-half rows 0:64 / B-half rows 64:128
  -> every ACT/DVE pointwise op runs with all 128 partitions busy.

x is converted to bf16 host-side and DMA'd directly into S1's x rows each
timestep (no staging copy).  All pointwise traffic is bf16 on VectorE
(2x TT mode, 4x copy mode); GPSIMD is not used for compute.
"""

import numpy as np

import concourse.bass as bass
import concourse.tile as tile
from concourse import bacc, mybir
from concourse.bass_utils import run_bass_kernel_spmd

F32 = mybir.dt.float32
BF16 = mybir.dt.bfloat16
AF = mybir.ActivationFunctionType

T, CIN, HID = 8, 28, 64
H = W = 128
HW = H * W            # pixels per core (one batch element)
NCORES = 8
K1, K2 = HID + CIN, 2 * HID   # S1 rows 0:64 = h1, 64:92 = x; S2 rows 0:64 = h1, 64:128 = h2

import os
CFG = dict(
    chunk=8192,        # pixels resident per chunk
    fd=2048,           # PSUM gate-tile free dim (pixels per half per block)
    nt=512,            # matmul moving tile (one PSUM bank of fp32)
    planes_bufs=1,     # chunk interleave gives cross-phase slack; 2 overflows SBUF
    pl_dtype="f32",    # gate/chain plane dtype: f32 ACT-writes ~30% faster
    c_dtype="bf16",    # cell-state dtype (bf16 frees SBUF for plane double-buffering)
    copy_mode="dma",   # h-row copies: "dma" (idle DMA engines) | "vector"
    t2_eng="vector",   # engine for t2 = si*tg (gpsimd measured 35% WORSE: chain latency + port contention)
    hp_eng="vector",   # engine for h = so*tanh(c)
)
for _k in list(CFG):
    _v = os.environ.get(f"KCFG_{_k.upper()}")
    if _v is not None:
        CFG[_k] = int(_v) if _v.isdigit() else _v


def _fold_weights(inputs):
    """Host-side exact algebraic folding (all fp32 numpy)."""
    f = np.float32
    W_red = inputs["W_red"].astype(f)
    b_red = inputs["b_red"].astype(f)
    # de-normalization of channels 11 (u) and 12 (v), folded into W_red
    a = np.ones(CIN, f); a[11] = f(0.15); a[12] = f(0.12)
    d = np.zeros(CIN, f); d[11] = f(0.02); d[12] = f(-0.01)
    W_red_eff = W_red * a[None, :]
    b_red_eff = b_red + W_red @ d

    W1x = inputs["Wih1"].astype(f) @ W_red_eff          # [256, 28]
    W1h = inputs["Whh1"].astype(f)                      # [256, 64]
    b1 = (inputs["bih1"] + inputs["bhh1"]).astype(f) + inputs["Wih1"].astype(f) @ b_red_eff
    W21 = inputs["Wih2"].astype(f) @ inputs["Wc1"].astype(f)   # [256, 64]
    W22 = inputs["Whh2"].astype(f)                      # [256, 64]
    b2 = (inputs["bih2"] + inputs["bhh2"]).astype(f) + inputs["Wih2"].astype(f) @ inputs["bc1"].astype(f)
    whead = (inputs["W_head"].astype(f) @ inputs["Wc2"].astype(f))[0]     # [64]
    bhead = float((inputs["W_head"].astype(f) @ inputs["bc2"].astype(f) + inputs["b_head"].astype(f)).reshape(()))

    w1 = np.ascontiguousarray(np.concatenate([W1h, W1x], axis=1).T)  # [92, 256]: h1 rows then x rows
    w2 = np.ascontiguousarray(np.concatenate([W21, W22], axis=1).T)  # [128, 256]
    # per-gate bias vectors duplicated across the two half-planes -> [128, 4]
    bdup = lambda b: np.stack([np.concatenate([b[64 * q:64 * q + 64]] * 2) for q in range(4)], axis=1)
    wh = np.zeros((128, 1), f); wh[64:, 0] = whead
    return dict(w1=w1, w2=w2, b1=np.ascontiguousarray(bdup(b1)),
                b2=np.ascontiguousarray(bdup(b2)), wh=wh,
                bh=np.full((128, 1), bhead, f))


def build(nc):
    chunk = CFG["chunk"]; fd = CFG["fd"]; nt = CFG["nt"]
    nchunk = HW // chunk
    half = chunk // 2
    nblk = half // fd
    nsub = fd // nt
    PL = {"f32": F32, "bf16": BF16}[CFG["pl_dtype"]]
    CD = {"f32": F32, "bf16": BF16}[CFG["c_dtype"]]

    def hcopy(dst, src):
        if CFG["copy_mode"] == "dma":
            nc.sync.dma_start(dst, src)
        else:
            nc.vector.tensor_copy(dst, src)

    t2_eng = getattr(nc, CFG["t2_eng"])
    hp_eng = getattr(nc, CFG["hp_eng"])

    x_d = nc.dram_tensor("xt", [T, CIN, HW], BF16, kind="ExternalInput").ap()
    w1_d = nc.dram_tensor("w1", [K1, 256], F32, kind="ExternalInput").ap()
    w2_d = nc.dram_tensor("w2", [K2, 256], F32, kind="ExternalInput").ap()
    wh_d = nc.dram_tensor("wh", [128, 1], F32, kind="ExternalInput").ap()
    b1_d = nc.dram_tensor("b1", [128, 4], F32, kind="ExternalInput").ap()
    b2_d = nc.dram_tensor("b2", [128, 4], F32, kind="ExternalInput").ap()
    bh_d = nc.dram_tensor("bh", [128, 1], F32, kind="ExternalInput").ap()
    # out[i, j] = pixel j*128 + i of this core's [H, W] map (host transposes)
    out_d = nc.dram_tensor("out", [128, HW // 128], F32, kind="ExternalOutput").ap()

    with tile.TileContext(nc) as tc:
        with (
            tc.tile_pool(name="const", bufs=1) as const,
            tc.tile_pool(name="state", bufs=1) as state,
            # ACT-destination planes need 2 bufs: the NEXT phase's ACTIVATE
            # must not wait for the lagging DVE chain to release the slot
            # (that WAR stall also idles the PE > the HAM window -> cold MMs)
            tc.tile_pool(name="actp", bufs=2) as actp,
            tc.tile_pool(name="chainp", bufs=2) as chainp,
            tc.tile_pool(name="outp", bufs=1) as outp,
            tc.tile_pool(name="psum", bufs=1, space=bass.MemorySpace.PSUM) as psum,
        ):
            # Stage weights via fp32 tiles + one convert copy each, so every
            # matmul waits on a single compute producer (direct weight DMA
            # measured 125us slower: the fused LDWEIGHTS has very few
            # sync-wait slots; multi-queue DMA deps overflow/serialize them).
            w1f = const.tile([K1, 256], F32, tag="w1f")
            w2f = const.tile([K2, 256], F32, tag="w2f")
            whf = const.tile([128, 1], F32, tag="whf")
            nc.sync.dma_start(w1f[:], w1_d)
            nc.sync.dma_start(w2f[:], w2_d)
            nc.sync.dma_start(whf[:], wh_d)
            w1_sb = const.tile([K1, 256], BF16, tag="w1")
            w2_sb = const.tile([K2, 256], BF16, tag="w2")
            wh_sb = const.tile([128, 1], BF16, tag="wh")
            nc.vector.tensor_copy(w1_sb[:], w1f[:])
            nc.vector.tensor_copy(w2_sb[:], w2f[:])
            nc.vector.tensor_copy(wh_sb[:], whf[:])
            b1_sb = const.tile([128, 4], F32, tag="b1")
            b2_sb = const.tile([128, 4], F32, tag="b2")
            bh_sb = const.tile([128, 1], F32, tag="bh")
            nc.sync.dma_start(b1_sb[:], b1_d)
            nc.sync.dma_start(b2_sb[:], b2_d)
            nc.sync.dma_start(bh_sb[:], bh_d)

            out_sb = outp.tile([128, HW // 128], F32, tag="osb")

            # dummy activation on an uninitialized tile (no DMA dep): hoists
            # the one-time ACT table load ahead of the first real gate
            warm = const.tile([128, 2], F32, tag="warm")
            nc.scalar.activation(warm[:, 1:2], warm[:, 0:1], AF.Sigmoid)
            # PE pre-warm: ~4.5us of dummy matmuls on garbage data during the
            # initial DMA window flips the HAM clock-gate to 8/8 (2.4 GHz)
            # before the first real gate matmuls issue
            wz = psum.tile([128, 128], F32, tag="P0", name="wz")
            for _ in range(40):
                nc.tensor.matmul(wz[0:1, :], out_sb[:, 0:1], out_sb[:, 0:128])

            # deferred post-chain: each (lst, last-block) post is flushed
            # after the NEXT phase's first gate ACT so the scalar queue
            # always has gate work to chew on during the DVE c-chain latency
            pending = []

            def flush_pending():
                while pending:
                    pending.pop(0)()

            # both chunks' recurrence state stays resident; their phases are
            # interleaved so every cross-phase dependency (c-chain latency,
            # h-row copies, x DMA WARs) hides behind the other chunk's work
            S1s, S2s, c1s, c2s = [], [], [], []
            for ci in range(nchunk):
                Sa = state.tile([K1, chunk], BF16, tag=f"S1_{ci}", name=f"S1_{ci}")
                Sb = state.tile([K2, chunk], BF16, tag=f"S2_{ci}", name=f"S2_{ci}")
                ca = state.tile([128, half], CD, tag=f"c1_{ci}", name=f"c1_{ci}")
                cb = state.tile([128, half], CD, tag=f"c2_{ci}", name=f"c2_{ci}")
                S1s.append(Sa); S2s.append(Sb); c1s.append(ca); c2s.append(cb)

            def emit_post(hb, cc, so, lst, t, S1, S2):
                # tanh(c) -> h -> scatter h rows into S1/S2.  The very last
                # posts (t=T-1, L2) gate the head + output: emit them in
                # quarter-blocks so the tanh/mul/copy links pipeline instead
                # of serializing at full width.
                nsp = 4 if (t == T - 1 and lst == 1) else 1
                sw = fd // nsp
                tch = actp.tile([128, fd], PL, tag="tc", name="tc")
                hp = actp.tile([128, fd], BF16, tag="hp", name="hp")
                for sp in range(nsp):
                    s0 = sp * sw
                    a0 = hb * fd + s0
                    b0 = half + hb * fd + s0
                    blk = slice(hb * fd + s0, hb * fd + s0 + sw)
                    nc.scalar.activation(tch[:, s0:s0 + sw], cc[:, blk], AF.Tanh)
                    hp_eng.tensor_mul(hp[:, s0:s0 + sw], so[:, s0:s0 + sw],
                                      tch[:, s0:s0 + sw])
                    if lst == 0:
                        # S2's h1 rows feed this step's L2 matmuls: first.
                        hcopy(S2[0:HID, a0:a0 + sw], hp[0:64, s0:s0 + sw])
                        hcopy(S2[0:HID, b0:b0 + sw], hp[64:128, s0:s0 + sw])
                        if t < T - 1:
                            # S1's h1 rows are only needed at t+1.
                            hcopy(S1[0:HID, a0:a0 + sw], hp[0:64, s0:s0 + sw])
                            hcopy(S1[0:HID, b0:b0 + sw], hp[64:128, s0:s0 + sw])
                    else:
                        hcopy(S2[HID:K2, a0:a0 + sw], hp[0:64, s0:s0 + sw])
                        hcopy(S2[HID:K2, b0:b0 + sw], hp[64:128, s0:s0 + sw])

            pp = 0
            for t in range(T):
                for ci in range(nchunk):
                    # x(t) straight into S1's x rows (host pre-converted bf16)
                    nc.sync.dma_start(S1s[ci][HID:K1, :],
                                      x_d[t][:, ci * chunk:(ci + 1) * chunk])
                for lst in (0, 1):
                    for ci in range(nchunk):
                        S1, S2 = S1s[ci], S2s[ci]
                        if lst == 0:
                            w_sb, b_sb, SS = w1_sb, b1_sb, S1
                            ks = slice(0, K1) if t > 0 else slice(HID, K1)
                            cc = c1s[ci]
                        else:
                            w_sb, b_sb, SS = w2_sb, b2_sb, S2
                            ks = slice(0, K2) if t > 0 else slice(0, HID)
                            cc = c2s[ci]

                        # per-block gate planes; ACT fills them from ping-pong
                        # PSUM tiles (2 x [128, fd] f32 = all 8 banks)
                        sos = [None] * nblk

                        for hb in range(nblk):
                            a0 = hb * fd            # A-half cols in S1/S2
                            b0 = half + hb * fd     # B-half cols
                            blk = slice(hb * fd, (hb + 1) * fd)
                            si = actp.tile([128, fd], PL, tag="si")
                            if t > 0:
                                sf = actp.tile([128, fd], PL, tag="sf")
                            else:
                                sf = None
                            tg = actp.tile([128, fd], PL, tag="tg")
                            so = actp.tile([128, fd], PL, tag="so")
                            sos[hb] = so
                            for qi, (pl, fn, q) in enumerate(
                                    ((si, AF.Sigmoid, 0), (sf, AF.Sigmoid, 1),
                                     (tg, AF.Tanh, 2), (so, AF.Sigmoid, 3))):
                                if pl is None:
                                    continue        # f-gate unused at t=0
                                P = psum.tile([128, fd], F32, tag=f"P{pp % 2}",
                                              name=f"P{pp % 2}")
                                pp += 1
                                for s in range(nsub):
                                    for (cb, po) in ((a0, 0), (b0, 64)):
                                        nc.tensor.matmul(
                                            P[po:po + 64, s * nt:(s + 1) * nt],
                                            w_sb[ks, q * 64:(q + 1) * 64],
                                            SS[ks, cb + s * nt:cb + (s + 1) * nt],
                                        )
                                nc.scalar.activation(pl[:], P[:], fn,
                                                     bias=b_sb[:, q:q + 1])
                                if qi == 0:
                                    # previous block's post-chain fills the ACT
                                    # queue behind this block's gates, so its
                                    # h rows land while gates still stream
                                    flush_pending()
                            if t > 0:
                                t1 = chainp.tile([128, fd], BF16, tag="t1")
                                t2 = chainp.tile([128, fd], BF16, tag="t2")
                                nc.vector.tensor_mul(t1[:], sf[:], cc[:, blk])
                                t2_eng.tensor_mul(t2[:], si[:], tg[:])
                                nc.vector.tensor_add(cc[:, blk], t1[:], t2[:])
                            else:
                                nc.vector.tensor_mul(cc[:, blk], si[:], tg[:])
                            pending.append(
                                lambda hb=hb, cc=cc, so=sos[hb],
                                       lst=lst, t=t, S1=S1, S2=S2:
                                    emit_post(hb, cc, so, lst, t, S1, S2))

            # head: out[pix] = whead @ h2[pix] + bh, pixels as matmul M-dim
            flush_pending()
            ncols = chunk // 128
            for ci in range(nchunk):
                ph = psum.tile([128, ncols], F32, tag="P0", name="ph")
                for j in range(ncols):
                    nc.tensor.matmul(
                        ph[:, j:j + 1],
                        S2s[ci][HID:K2, j * 128:(j + 1) * 128],
                        wh_sb[64:128, 0:1],
                    )
                nc.vector.tensor_scalar_add(
                    out_sb[:, ci * ncols:(ci + 1) * ncols], ph[:], bh_sb[:, 0:1])
                nc.sync.dma_start(out_d[:, ci * ncols:(ci + 1) * ncols],
                                  out_sb[:, ci * ncols:(ci + 1) * ncols])
    nc.compile()
    return nc


def _make_nc():
    # Bacc (not raw Bass): its compile() runs move_matmul_waits_to_ldweights +
    # generate_event_semaphores, required to satisfy TRN2's 1-wait-per-inst limit.
    return bacc.Bacc("TRN2", target_bir_lowering=False, debug=False,
                     num_devices=NCORES, enable_partition_id=False)


def _to_bf16(a):
    import ml_dtypes
    return a.astype(ml_dtypes.bfloat16)


def _in_maps(inputs):
    folded = _fold_weights(inputs)
    x = np.asarray(inputs["x"], dtype=np.float32)
    maps = []
    for b in range(NCORES):
        m = dict(folded)
        m["xt"] = _to_bf16(np.ascontiguousarray(x[b].reshape(T, CIN, HW)))
        maps.append(m)
    return maps


def _assemble(results):
    out = np.empty((NCORES, H, W), np.float32)
    for b in range(NCORES):
        o = results[b]["out"]          # [128, HW//128], o[i, j] = pixel j*128+i
        out[b] = o.T.reshape(H, W)
    return out


def _run(inputs, trace=False):
    nc = build(_make_nc())
    maps = _in_maps(inputs)
    res = run_bass_kernel_spmd(nc, maps, core_ids=list(range(NCORES)), trace=trace)
    return _assemble(res.results), res


def kernel(**inputs) -> np.ndarray:
    out, _ = _run(inputs, trace=False)
    return out
